# revision 21
# baseline (speedup 1.0000x reference)
"""Causal self-attention (ALiBi + QK-RMSNorm + subln) on 8 TRN2 NeuronCores.

Sharding: 8 cores = 2 batches x 4 head-groups (4 heads / 128 head-dim each).
Per core: QKV projection for its 512 features, attention for its 4 heads,
and a partial output projection (row slice of Wproj); host sums the 4
partials per batch.

All matmuls run as float32r (full-rate fp32 streaming with ~11-bit input
rounding, fp32 accumulation).
"""
import math

import numpy as np

import concourse.bacc as bacc
import concourse.bass as bass
import concourse.mybir as mybir
from concourse.tile import TileContext

F32 = mybir.dt.float32
F32R = mybir.dt.float32r
AF = mybir.ActivationFunctionType

B, T, C = 2, 2048, 2048
TC_G = 4
H, D = 16, 128
HG = 4          # head groups = cores per batch
HPG = 4         # heads per group
F = HPG * D     # 512 per-core qkv features
EPS = 1e-5
NEG = -1.0e30
PIN = 64        # diagonal pinning offset for ALiBi exp factorization


def _alibi_slopes(n_heads):
    def pow2(n):
        start = 2 ** (-(2 ** (-(math.log2(n) - 3))))
        return [start * start**i for i in range(n)]

    if math.log2(n_heads).is_integer():
        return pow2(n_heads)
    c = 2 ** math.floor(math.log2(n_heads))
    s = pow2(c)
    extra = _alibi_slopes(2 * c)
    return s + extra[0::2][: n_heads - c]


def _build(debug=False):
    nc = bacc.Bacc("TRN2", target_bir_lowering=False)

    xt = nc.dram_tensor("xt", [C, T], F32R, kind="ExternalInput")
    wq = nc.dram_tensor("wq", [C, F], F32R, kind="ExternalInput")
    wk = nc.dram_tensor("wk", [C, F], F32R, kind="ExternalInput")
    wv = nc.dram_tensor("wv", [C, F], F32R, kind="ExternalInput")
    wp = nc.dram_tensor("wp", [F, C], F32R, kind="ExternalInput")
    bias_tab = nc.dram_tensor("bias_tab", [128, HPG * 16], F32, kind="ExternalInput")
    cmask = nc.dram_tensor("cmask", [128, 128], F32, kind="ExternalInput")
    wqk = nc.dram_tensor("wqk", [128, 1], F32, kind="ExternalInput")
    ones_c = nc.dram_tensor("ones_c", [128, 1], F32R, kind="ExternalInput")
    ones_r = nc.dram_tensor("ones_r", [1, 128], F32R, kind="ExternalInput")
    dcol = nc.dram_tensor("dcol", [128, HPG * 16], F32R, kind="ExternalInput")
    out = nc.dram_tensor("out", [T, C], F32, kind="ExternalOutput")

    kind_s = dict(kind="ExternalOutput") if debug else {}
    qt_s = nc.dram_tensor("qt_s", [F, T], F32R, **kind_s)
    kt_s = nc.dram_tensor("kt_s", [F, T], F32R, **kind_s)
    v_s = nc.dram_tensor("v_s", [T, F], F32R, **kind_s)
    yfin_d = nc.dram_tensor("yfin_d", [F, T], F32, kind="ExternalOutput") if debug else None
    den_d = nc.dram_tensor("den_d", [HPG * TC_G, 512], F32, kind="ExternalOutput") if debug else None

    TC = T // 512  # 4 t-chunks

    with nc.allow_low_precision(reason="f32r rounding of matmul operands is intentional"), TileContext(nc) as tc:
        with (
            tc.tile_pool(name="consts", bufs=1) as consts,
            tc.tile_pool(name="psum", bufs=1, space="PSUM") as psum,
        ):
            bias_t = consts.tile([128, HPG * 16], F32, tag="bias_t")
            nc.sync.dma_start(out=bias_t, in_=bias_tab[:, :])
            mask_t = consts.tile([128, 128], F32, tag="mask_t")
            nc.sync.dma_start(out=mask_t, in_=cmask[:, :])
            wqk_t = consts.tile([128, 1], F32, tag="wqk_t")
            nc.sync.dma_start(out=wqk_t, in_=wqk[:, :])
            onesc_t = consts.tile([128, 1], F32R, tag="onesc_t")
            nc.sync.dma_start(out=onesc_t, in_=ones_c[:, :])
            onesr_t = consts.tile([1, 128], F32R, tag="onesr_t")
            nc.sync.dma_start(out=onesr_t, in_=ones_r[:, :])
            dcol_t = consts.tile([128, HPG * 16], F32R, tag="dcol_t")
            nc.sync.dma_start(out=dcol_t, in_=dcol[:, :])
            eps_c = consts.tile([128, 1], F32, tag="eps_c")
            nc.vector.memset(eps_c, EPS)
            eps_r = consts.tile([1, 1], F32, tag="eps_r")
            nc.vector.memset(eps_r, EPS)
            eps128_r = consts.tile([1, 1], F32, tag="eps128_r")
            nc.vector.memset(eps128_r, 128.0 * EPS)
            ones_f = consts.tile([128, 256], F32, tag="ones_f")
            nc.vector.memset(ones_f, 1.0)
            ones_m = consts.tile([128, 256], F32R, tag="ones_m")
            nc.vector.tensor_copy(ones_m, ones_f)
            zeros_f = consts.tile([128, 384], F32, tag="zeros_f")
            nc.vector.memset(zeros_f, 0.0)
            zeros_r = consts.tile([128, 384], F32R, tag="zeros_r")
            nc.vector.tensor_copy(zeros_r, zeros_f)

            # rk columns for all (head, j-tile), filled during phase A
            rk_all = consts.tile([128, HPG * 16], F32, tag="rk_all")

            # ---------------- Phase A: QKV (+ q/k norm) -> DRAM scratch ---
            with (
                tc.tile_pool(name="wpool", bufs=1) as wpool,
                tc.tile_pool(name="xpool", bufs=2) as xpool,
                tc.tile_pool(name="stg", bufs=2) as stg,
            ):
                wq_t = wpool.tile([128, 16, F], F32R, tag="wq_t")
                nc.sync.dma_start(out=wq_t, in_=wq.rearrange("(ct p) f -> p ct f", p=128))
                wk_t = wpool.tile([128, 16, F], F32R, tag="wk_t")
                nc.sync.dma_start(out=wk_t, in_=wk.rearrange("(ct p) f -> p ct f", p=128))
                wv_t = wpool.tile([128, 16, F], F32R, tag="wv_t")
                nc.sync.dma_start(out=wv_t, in_=wv.rearrange("(ct p) f -> p ct f", p=128))

                xt_r = xt.rearrange("(ct p) t -> p ct t", p=128)
                for tch in range(TC):
                    x_t = xpool.tile([128, 16, 512], F32R, tag="x_t")
                    nc.sync.dma_start(out=x_t, in_=xt_r[:, :, tch * 512:(tch + 1) * 512])

                    # ---- q: project, rms-normalize (1/sqrt(D) folded), store
                    for ft in range(4):
                        ps = psum.tile([128, 512], F32, tag="big_ps", bufs=2)
                        for ct in range(16):
                            nc.tensor.matmul(
                                ps,
                                wq_t[:, ct, ft * 128:(ft + 1) * 128],
                                x_t[:, ct, :],
                                start=(ct == 0),
                                stop=(ct == 15),
                            )
                        st = stg.tile([128, 512], F32R, tag="st", bufs=3)
                        nc.scalar.copy(st, ps)
                        qsq = stg.tile([128, 512], F32R, tag="qsq")
                        nc.vector.tensor_mul(qsq, st.bitcast(F32), st.bitcast(F32))
                        ps_row = psum.tile([1, 512], F32, tag="rowb_ps", bufs=2)
                        nc.tensor.matmul(ps_row, onesc_t, qsq, start=True, stop=True)
                        rq_f = stg.tile([1, 512], F32, tag="rq_f")
                        nc.scalar.activation(
                            rq_f, ps_row, AF.Sqrt, scale=1.0, bias=eps128_r
                        )
                        rq_f2 = stg.tile([1, 512], F32, tag="rq_f2")
                        nc.vector.reciprocal_approx_fast(rq_f2, rq_f)
                        rq_row = stg.tile([1, 512], F32R, tag="rq_row")
                        nc.vector.tensor_copy(rq_row, rq_f2)
                        ps_b = psum.tile([128, 512], F32, tag="rowb_ps", bufs=2)
                        nc.tensor.matmul(ps_b, onesr_t, rq_row, start=True, stop=True)
                        qhat = stg.tile([128, 512], F32R, tag="qhat", bufs=3)
                        nc.vector.tensor_mul(qhat, st.bitcast(F32), ps_b)
                        nc.sync.dma_start(
                            out=qt_s[ft * 128:(ft + 1) * 128,
                                     tch * 512:(tch + 1) * 512],
                            in_=qhat,
                        )

                    # ---- k: project, fold wq*wk, rk columns, store
                    for ft in range(4):
                        ps = psum.tile([128, 512], F32, tag="big_ps", bufs=2)
                        for ct in range(16):
                            nc.tensor.matmul(
                                ps,
                                wk_t[:, ct, ft * 128:(ft + 1) * 128],
                                x_t[:, ct, :],
                                start=(ct == 0),
                                stop=(ct == 15),
                            )
                        st = stg.tile([128, 512], F32R, tag="st", bufs=3)
                        nc.scalar.copy(st, ps)
                        ksq = stg.tile([128, 512], F32R, tag="ksq")
                        nc.vector.tensor_mul(ksq, st.bitcast(F32), st.bitcast(F32))
                        for ts4 in range(4):
                            jt = tch * 4 + ts4
                            psk = psum.tile([128, 256], F32, tag="s_ps", bufs=2)
                            nc.tensor.matmul(
                                psk, ksq[:, ts4 * 128:(ts4 + 1) * 128], ones_m,
                                start=True, stop=True,
                            )
                            col = rk_all[:, ft * 16 + jt:ft * 16 + jt + 1]
                            nc.scalar.activation(
                                col, psk[:, 0:1], AF.Sqrt,
                                scale=1.0 / 128.0, bias=eps_c,
                            )
                            nc.vector.reciprocal(col, col)
                        khat = stg.tile([128, 512], F32R, tag="khat_a", bufs=3)
                        nc.vector.tensor_scalar_mul(
                            khat, st.bitcast(F32), scalar1=wqk_t
                        )
                        nc.sync.dma_start(
                            out=kt_s[ft * 128:(ft + 1) * 128,
                                     tch * 512:(tch + 1) * 512],
                            in_=khat,
                        )

                    # ---- v: natural layout, store
                    for ts4 in range(4):
                        ps = psum.tile([128, 512], F32, tag="big_ps", bufs=2)
                        for ct in range(16):
                            nc.tensor.matmul(
                                ps,
                                x_t[:, ct, ts4 * 128:(ts4 + 1) * 128],
                                wv_t[:, ct, :],
                                start=(ct == 0),
                                stop=(ct == 15),
                            )
                        st = stg.tile([128, 512], F32R, tag="st", bufs=3)
                        nc.scalar.copy(st, ps)
                        nc.sync.dma_start(
                            out=v_s[(tch * 4 + ts4) * 128:(tch * 4 + ts4 + 1) * 128, :],
                            in_=st,
                        )

            # ---------------- Phase B: attention per head -----------------
            with (
                tc.tile_pool(name="head", bufs=1) as head,
                tc.tile_pool(name="ppool", bufs=3) as ppool,
                tc.tile_pool(name="yfin_pool", bufs=1) as yfin_pool,
                tc.tile_pool(name="small", bufs=2) as small,
            ):
                yfin = []
                for h in range(HPG):
                    yf = yfin_pool.tile([128, T], F32R, tag=f"yfin{h}")
                    yfin.append(yf)

                v_sr = v_s.rearrange("(jt p) f -> p jt f", p=128)
                ysum_sb = []  # [1,512] per (h, ic) for deferred subln rstd
                for idx in range(HPG * TC):
                    yt = small.tile([1, 512], F32R, tag=f"ysum{idx}", bufs=1)
                    ysum_sb.append(yt)

                def finish_chunk(y_ps, den_ps, h, ic):
                    # yn = y_unnorm / den  (done via bcast + 128-lane recip);
                    # rstd is deferred to the batched pass after attention.
                    dsb = small.tile([1, 512], F32R, tag="dsb")
                    nc.scalar.copy(dsb, den_ps)
                    db_ps = psum.tile([128, 512], F32, tag="rowb_ps", bufs=2)
                    nc.tensor.matmul(db_ps, onesr_t, dsb, start=True, stop=True)
                    recip_b = small.tile([128, 512], F32, tag="recip_b")
                    nc.vector.reciprocal_approx_fast(recip_b, db_ps)
                    yslice = yfin[h][:, ic * 512:(ic + 1) * 512]
                    nc.vector.tensor_mul(yslice, y_ps, recip_b)
                    ysq = small.tile([128, 512], F32R, tag="ysq")
                    nc.vector.tensor_mul(
                        ysq, yslice.bitcast(F32), yslice.bitcast(F32)
                    )
                    ysum_ps = psum.tile([1, 512], F32, tag="rowb_ps", bufs=2)
                    nc.tensor.matmul(ysum_ps, onesc_t, ysq, start=True, stop=True)
                    nc.scalar.copy(ysum_sb[h * TC + ic], ysum_ps)

                def emit_head_chunk_prep(h, ic, khat_by_h, v_by_h):
                    qhat = small.tile([128, 512], F32R, tag="qhat", bufs=4)
                    nc.sync.dma_start(
                        out=qhat,
                        in_=qt_s[h * 128:(h + 1) * 128,
                                 ic * 512:(ic + 1) * 512],
                    )
                    njt = 4 * ic + 4
                    single_exp = h >= 2
                    vhat = None
                    if single_exp:
                        vhat = head.tile([128, 16, 128], F32R, tag="vhat",
                                         bufs=2)
                        for jt in range(njt):
                            kk = njt - 1 - jt
                            nc.vector.tensor_scalar_mul(
                                vhat[:, jt, :],
                                v_by_h[h][:, jt, :].bitcast(F32),
                                scalar1=dcol_t[:, h * 16 + kk:
                                               h * 16 + kk + 1].bitcast(F32),
                            )
                    y_ps = psum.tile([128, 512], F32, tag="big_ps", bufs=2)
                    den_ps = psum.tile([1, 512], F32, tag="den_ps", bufs=2)
                    return dict(h=h, ic=ic, qhat=qhat, vhat=vhat, y_ps=y_ps,
                                den_ps=den_ps, njt=njt, single_exp=single_exp,
                                khat=khat_by_h[h], v_h=v_by_h[h])

                def emit_jt_iter(st, jt):
                    h, ic = st["h"], st["ic"]
                    rk = rk_all[:, h * 16:(h + 1) * 16]
                    njt = st["njt"]
                    s_ps = psum.tile([128, 512], F32, tag="s_ps", bufs=2)
                    nc.tensor.matmul(
                        s_ps, st["khat"][:, jt * 128:(jt + 1) * 128],
                        st["qhat"], start=True, stop=True,
                    )
                    pt = ppool.tile([128, 512], F32R, tag="pt")
                    i_lo = max(0, jt - 4 * ic)
                    if st["single_exp"]:
                        if i_lo > 0:
                            nc.vector.tensor_scalar_add(
                                s_ps[:, 0:i_lo * 128],
                                s_ps[:, 0:i_lo * 128], scalar1=NEG,
                            )
                        if jt >= 4 * ic:
                            isub = jt - 4 * ic
                            src_b = s_ps[:, isub * 128:(isub + 1) * 128]
                            nc.vector.tensor_add(src_b, src_b, mask_t)
                        nc.scalar.activation(
                            pt, s_ps, AF.Exp,
                            scale=rk[:, jt:jt + 1],
                            bias=bias_t[:, h * 16:h * 16 + 1],
                        )
                        kk = njt - 1 - jt
                        av_lhs = st["vhat"][:, jt, :]
                        den_lhs = dcol_t[:, h * 16 + kk:h * 16 + kk + 1]
                    else:
                        if i_lo > 0:
                            nc.vector.tensor_copy(
                                pt[:, 0:i_lo * 128], zeros_r[:, 0:i_lo * 128]
                            )
                        for isub in range(i_lo, 4):
                            diff = 4 * ic + isub - jt
                            src_b = s_ps[:, isub * 128:(isub + 1) * 128]
                            if diff == 0:
                                nc.vector.tensor_add(src_b, src_b, mask_t)
                            nc.scalar.activation(
                                pt[:, isub * 128:(isub + 1) * 128], src_b,
                                AF.Exp,
                                scale=rk[:, jt:jt + 1],
                                bias=bias_t[:, h * 16 + diff:
                                            h * 16 + diff + 1],
                            )
                        av_lhs = st["v_h"][:, jt, :]
                        den_lhs = onesc_t
                    nc.tensor.matmul(
                        st["y_ps"], av_lhs, pt,
                        start=(jt == 0), stop=(jt == njt - 1),
                        skip_group_check=True,
                    )
                    nc.tensor.matmul(
                        st["den_ps"], den_lhs, pt,
                        start=(jt == 0), stop=(jt == njt - 1),
                        skip_group_check=True,
                    )

                # pair a 4-exp head (0/1) with a 1-exp head (2/3): the
                # interleaved streams keep both PE and ACT busy
                khat_by_h = {}
                v_by_h = {}
                for pair in ((0, 2), (1, 3)):
                    for h in pair:
                        kh = head.tile([128, T], F32R, tag="khat", bufs=3)
                        nc.sync.dma_start(
                            out=kh, in_=kt_s[h * 128:(h + 1) * 128, :]
                        )
                        khat_by_h[h] = kh
                        vh = head.tile([128, 16, 128], F32R, tag="v_h", bufs=3)
                        nc.sync.dma_start(
                            out=vh, in_=v_sr[:, :, h * 128:(h + 1) * 128]
                        )
                        v_by_h[h] = vh
                    for ic in range(TC):
                        sa = emit_head_chunk_prep(pair[0], ic, khat_by_h, v_by_h)
                        sb = emit_head_chunk_prep(pair[1], ic, khat_by_h, v_by_h)
                        for jt in range(4 * ic + 4):
                            emit_jt_iter(sa, jt)
                            emit_jt_iter(sb, jt)
                        finish_chunk(sa["y_ps"], sa["den_ps"], pair[0], ic)
                        finish_chunk(sb["y_ps"], sb["den_ps"], pair[1], ic)

                # batched subln rstd: one table-switch total, 128-lane recips
                for h in range(HPG):
                    for ic in range(TC):
                        ysb_ps = psum.tile([128, 512], F32, tag="rowb_ps", bufs=2)
                        nc.tensor.matmul(
                            ysb_ps, onesr_t, ysum_sb[h * TC + ic],
                            start=True, stop=True,
                        )
                        srt = small.tile([128, 512], F32, tag="srt")
                        nc.scalar.activation(
                            srt, ysb_ps, AF.Sqrt, scale=1.0 / 128.0, bias=eps_c
                        )
                        srt2 = small.tile([128, 512], F32, tag="srt2")
                        nc.vector.reciprocal_approx_fast(srt2, srt)
                        yslice = yfin[h][:, ic * 512:(ic + 1) * 512]
                        nc.vector.tensor_mul(yslice, yslice.bitcast(F32), srt2)

                # ------------- Phase C: output projection -----------------
                with (
                    tc.tile_pool(name="wppool", bufs=1) as wppool,
                    tc.tile_pool(name="opool", bufs=3) as opool,
                ):
                    wp_t = wppool.tile([128, HPG, C], F32R, tag="wp_t")
                    nc.sync.dma_start(
                        out=wp_t, in_=wp.rearrange("(ht p) c -> p ht c", p=128)
                    )
                    for tt in range(16):
                        for cc in range(4):
                            ps = psum.tile([128, 512], F32, tag="big_ps", bufs=2)
                            for h in range(HPG):
                                nc.tensor.matmul(
                                    ps,
                                    yfin[h][:, tt * 128:(tt + 1) * 128],
                                    wp_t[:, h, cc * 512:(cc + 1) * 512],
                                    start=(h == 0),
                                    stop=(h == HPG - 1),
                                )
                            ot = opool.tile([128, 512], F32, tag="ot")
                            nc.vector.tensor_copy(ot, ps)
                            nc.sync.dma_start(
                                out=out[tt * 128:(tt + 1) * 128,
                                        cc * 512:(cc + 1) * 512],
                                in_=ot,
                            )

    nc.compile()
    return nc


_NC_CACHE = None


def _get_nc():
    global _NC_CACHE
    if _NC_CACHE is None:
        _NC_CACHE = _build()
    return _NC_CACHE


def kernel_in_maps(x, Wq, Wk, Wv, Wproj, q_rms_w, k_rms_w, subln_w):
    slopes = _alibi_slopes(H)

    x = np.asarray(x, dtype=np.float32)
    Wq = np.asarray(Wq, dtype=np.float32)
    Wk = np.asarray(Wk, dtype=np.float32)
    Wv = np.asarray(Wv, dtype=np.float32)
    Wproj = np.asarray(Wproj, dtype=np.float32)
    q_rms_w = np.asarray(q_rms_w, dtype=np.float32)
    k_rms_w = np.asarray(k_rms_w, dtype=np.float32)
    subln_w = np.asarray(subln_w, dtype=np.float32)

    wqk = (q_rms_w * k_rms_w).reshape(128, 1)
    cmask = np.where(
        np.arange(128)[:, None] <= np.arange(128)[None, :], 0.0, NEG
    ).astype(np.float32)
    ones_c = np.ones((128, 1), np.float32)
    ones_r = np.ones((1, 128), np.float32)
    dj = np.arange(128, dtype=np.float32)

    in_maps = []
    for b in range(B):
        xt = np.ascontiguousarray(x[b].T)
        for g in range(HG):
            heads = [g + 4 * j for j in range(HPG)]  # strided: slopes shrink with j
            csel = np.concatenate(
                [np.arange(hh * D, (hh + 1) * D) for hh in heads]
            )
            wproj_s = np.ascontiguousarray(
                Wproj[csel, :] * np.tile(subln_w, HPG)[:, None]
            )
            bias_tab = np.empty((128, HPG * 16), np.float32)
            dcol_a = np.empty((128, HPG * 16), np.float32)
            for j, hh in enumerate(heads):
                slope = slopes[hh]
                for diff in range(16):
                    bias_tab[:, j * 16 + diff] = slope * (dj - PIN - 128.0 * diff)
                    dcol_a[:, j * 16 + diff] = np.float32(
                        math.exp(-slope * 128.0 * diff) if slope * 128.0 * diff < 700
                        else 0.0
                    )
            in_maps.append({
                "xt": xt,
                "wq": np.ascontiguousarray(Wq[:, csel]),
                "wk": np.ascontiguousarray(Wk[:, csel]),
                "wv": np.ascontiguousarray(Wv[:, csel]),
                "wp": wproj_s,
                "bias_tab": bias_tab,
                "cmask": cmask,
                "wqk": wqk,
                "ones_c": ones_c,
                "ones_r": ones_r,
                "dcol": dcol_a,
            })

    return in_maps


def gather(results):
    outs = [r["out"] for r in results]
    final = np.stack(
        [sum(outs[b * HG + 1:(b + 1) * HG], outs[b * HG]) for b in range(B)]
    )
    return final.astype(np.float32)


def kernel(x, Wq, Wk, Wv, Wproj, q_rms_w, k_rms_w, subln_w):
    from concourse.bass_utils import run_bass_kernel_spmd

    in_maps = kernel_in_maps(x, Wq, Wk, Wv, Wproj, q_rms_w, k_rms_w, subln_w)
    res = run_bass_kernel_spmd(_get_nc(), in_maps, core_ids=list(range(8)))
    return gather(res.results)


if __name__ == "__main__":
    rng = np.random.default_rng(0)
    ins = {
        "x": rng.standard_normal((B, T, C), dtype=np.float32),
        "Wq": rng.standard_normal((C, H * D), dtype=np.float32) / math.sqrt(C),
        "Wk": rng.standard_normal((C, H * D), dtype=np.float32) / math.sqrt(C),
        "Wv": rng.standard_normal((C, H * D), dtype=np.float32) / math.sqrt(C),
        "Wproj": rng.standard_normal((H * D, C), dtype=np.float32) * 0.001,
        "q_rms_w": np.ones(D, np.float32),
        "k_rms_w": np.ones(D, np.float32),
        "subln_w": np.ones(D, np.float32),
    }
    y = kernel(**ins)
    print("kernel output", y.shape, y.dtype, float(np.abs(y).mean()))


# revision 22
# speedup vs baseline: 1.0484x; 1.0484x over previous
"""Causal self-attention (ALiBi + QK-RMSNorm + subln) on 8 TRN2 NeuronCores.

Sharding: 8 cores = 2 batches x 4 head-groups (4 heads / 128 head-dim each).
Per core: QKV projection for its 512 features, attention for its 4 heads,
and a partial output projection (row slice of Wproj); host sums the 4
partials per batch.

All matmuls run as float32r (full-rate fp32 streaming with ~11-bit input
rounding, fp32 accumulation).
"""
import math

import numpy as np

import concourse.bacc as bacc
import concourse.bass as bass
import concourse.mybir as mybir
from concourse.tile import TileContext

F32 = mybir.dt.float32
F32R = mybir.dt.float32r
AF = mybir.ActivationFunctionType

B, T, C = 2, 2048, 2048
TC_G = 4
H, D = 16, 128
HG = 4          # head groups = cores per batch
HPG = 4         # heads per group
F = HPG * D     # 512 per-core qkv features
EPS = 1e-5
NEG = -1.0e30
PIN = 64        # diagonal pinning offset for ALiBi exp factorization


def _alibi_slopes(n_heads):
    def pow2(n):
        start = 2 ** (-(2 ** (-(math.log2(n) - 3))))
        return [start * start**i for i in range(n)]

    if math.log2(n_heads).is_integer():
        return pow2(n_heads)
    c = 2 ** math.floor(math.log2(n_heads))
    s = pow2(c)
    extra = _alibi_slopes(2 * c)
    return s + extra[0::2][: n_heads - c]


def _build(debug=False):
    nc = bacc.Bacc("TRN2", target_bir_lowering=False)

    xt = nc.dram_tensor("xt", [C, T], F32R, kind="ExternalInput")
    wq = nc.dram_tensor("wq", [C, F], F32R, kind="ExternalInput")
    wk = nc.dram_tensor("wk", [C, F], F32R, kind="ExternalInput")
    wv = nc.dram_tensor("wv", [C, F], F32R, kind="ExternalInput")
    wp = nc.dram_tensor("wp", [F, C], F32R, kind="ExternalInput")
    bias_tab = nc.dram_tensor("bias_tab", [128, HPG * 16], F32, kind="ExternalInput")
    cmask = nc.dram_tensor("cmask", [128, 128], F32, kind="ExternalInput")
    wqk = nc.dram_tensor("wqk", [128, 1], F32, kind="ExternalInput")
    ones_c = nc.dram_tensor("ones_c", [128, 1], F32R, kind="ExternalInput")
    ones_r = nc.dram_tensor("ones_r", [1, 128], F32R, kind="ExternalInput")
    dcol = nc.dram_tensor("dcol", [128, HPG * 16], F32R, kind="ExternalInput")
    out = nc.dram_tensor("out", [T, C], F32, kind="ExternalOutput")

    kind_s = dict(kind="ExternalOutput") if debug else {}
    qt_s = nc.dram_tensor("qt_s", [F, T], F32R, **kind_s)
    kt_s = nc.dram_tensor("kt_s", [F, T], F32R, **kind_s)
    v_s = nc.dram_tensor("v_s", [T, F], F32R, **kind_s)
    yfin_d = nc.dram_tensor("yfin_d", [F, T], F32, kind="ExternalOutput") if debug else None
    den_d = nc.dram_tensor("den_d", [HPG * TC_G, 512], F32, kind="ExternalOutput") if debug else None

    TC = T // 512  # 4 t-chunks

    with nc.allow_low_precision(reason="f32r rounding of matmul operands is intentional"), TileContext(nc) as tc:
        with (
            tc.tile_pool(name="consts", bufs=1) as consts,
            tc.tile_pool(name="psum", bufs=1, space="PSUM") as psum,
        ):
            bias_t = consts.tile([128, HPG * 16], F32, tag="bias_t")
            nc.sync.dma_start(out=bias_t, in_=bias_tab[:, :])
            mask_t = consts.tile([128, 128], F32, tag="mask_t")
            nc.sync.dma_start(out=mask_t, in_=cmask[:, :])
            wqk_t = consts.tile([128, 1], F32, tag="wqk_t")
            nc.sync.dma_start(out=wqk_t, in_=wqk[:, :])
            onesc_t = consts.tile([128, 1], F32R, tag="onesc_t")
            nc.sync.dma_start(out=onesc_t, in_=ones_c[:, :])
            onesr_t = consts.tile([1, 128], F32R, tag="onesr_t")
            nc.sync.dma_start(out=onesr_t, in_=ones_r[:, :])
            dcol_t = consts.tile([128, HPG * 16], F32R, tag="dcol_t")
            nc.sync.dma_start(out=dcol_t, in_=dcol[:, :])
            eps_c = consts.tile([128, 1], F32, tag="eps_c")
            nc.vector.memset(eps_c, EPS)
            eps_r = consts.tile([1, 1], F32, tag="eps_r")
            nc.vector.memset(eps_r, EPS)
            eps128_r = consts.tile([1, 1], F32, tag="eps128_r")
            nc.vector.memset(eps128_r, 128.0 * EPS)
            ones_f = consts.tile([128, 256], F32, tag="ones_f")
            nc.vector.memset(ones_f, 1.0)
            ones_m = consts.tile([128, 256], F32R, tag="ones_m")
            nc.vector.tensor_copy(ones_m, ones_f)
            zeros_f = consts.tile([128, 384], F32, tag="zeros_f")
            nc.vector.memset(zeros_f, 0.0)
            zeros_r = consts.tile([128, 384], F32R, tag="zeros_r")
            nc.vector.tensor_copy(zeros_r, zeros_f)

            # rk columns for all (head, j-tile), filled during phase A
            rk_all = consts.tile([128, HPG * 16], F32, tag="rk_all")

            # ---------------- Phase A: QKV (+ q/k norm) -> DRAM scratch ---
            with (
                tc.tile_pool(name="wpool", bufs=1) as wpool,
                tc.tile_pool(name="xpool", bufs=2) as xpool,
                tc.tile_pool(name="stg", bufs=2) as stg,
            ):
                wq_t = wpool.tile([128, 16, F], F32R, tag="wq_t")
                nc.sync.dma_start(out=wq_t, in_=wq.rearrange("(ct p) f -> p ct f", p=128))
                wk_t = wpool.tile([128, 16, F], F32R, tag="wk_t")
                nc.sync.dma_start(out=wk_t, in_=wk.rearrange("(ct p) f -> p ct f", p=128))
                wv_t = wpool.tile([128, 16, F], F32R, tag="wv_t")
                nc.sync.dma_start(out=wv_t, in_=wv.rearrange("(ct p) f -> p ct f", p=128))

                xt_r = xt.rearrange("(ct p) t -> p ct t", p=128)
                for tch in range(TC):
                    x_t = xpool.tile([128, 16, 512], F32R, tag="x_t")
                    nc.sync.dma_start(out=x_t, in_=xt_r[:, :, tch * 512:(tch + 1) * 512])

                    # ---- q: project, rms-normalize (1/sqrt(D) folded), store
                    for ft in range(4):
                        ps = psum.tile([128, 512], F32, tag="big_ps", bufs=2)
                        for ct in range(16):
                            nc.tensor.matmul(
                                ps,
                                wq_t[:, ct, ft * 128:(ft + 1) * 128],
                                x_t[:, ct, :],
                                start=(ct == 0),
                                stop=(ct == 15),
                            )
                        st = stg.tile([128, 512], F32R, tag="st", bufs=3)
                        nc.scalar.copy(st, ps)
                        qsq = stg.tile([128, 512], F32R, tag="qsq")
                        nc.vector.tensor_mul(qsq, st.bitcast(F32), st.bitcast(F32))
                        ps_row = psum.tile([1, 512], F32, tag="rowb_ps", bufs=2)
                        nc.tensor.matmul(ps_row, onesc_t, qsq, start=True, stop=True)
                        rq_f = stg.tile([1, 512], F32, tag="rq_f")
                        nc.scalar.activation(
                            rq_f, ps_row, AF.Sqrt, scale=1.0, bias=eps128_r
                        )
                        rq_f2 = stg.tile([1, 512], F32, tag="rq_f2")
                        nc.vector.reciprocal_approx_fast(rq_f2, rq_f)
                        rq_row = stg.tile([1, 512], F32R, tag="rq_row")
                        nc.vector.tensor_copy(rq_row, rq_f2)
                        ps_b = psum.tile([128, 512], F32, tag="rowb_ps", bufs=2)
                        nc.tensor.matmul(ps_b, onesr_t, rq_row, start=True, stop=True)
                        qhat = stg.tile([128, 512], F32R, tag="qhat", bufs=3)
                        nc.vector.tensor_mul(qhat, st.bitcast(F32), ps_b)
                        nc.sync.dma_start(
                            out=qt_s[ft * 128:(ft + 1) * 128,
                                     tch * 512:(tch + 1) * 512],
                            in_=qhat,
                        )

                    # ---- k: project, fold wq*wk, rk columns, store
                    for ft in range(4):
                        ps = psum.tile([128, 512], F32, tag="big_ps", bufs=2)
                        for ct in range(16):
                            nc.tensor.matmul(
                                ps,
                                wk_t[:, ct, ft * 128:(ft + 1) * 128],
                                x_t[:, ct, :],
                                start=(ct == 0),
                                stop=(ct == 15),
                            )
                        st = stg.tile([128, 512], F32R, tag="st", bufs=3)
                        nc.scalar.copy(st, ps)
                        ksq = stg.tile([128, 512], F32R, tag="ksq")
                        nc.vector.tensor_mul(ksq, st.bitcast(F32), st.bitcast(F32))
                        for ts4 in range(4):
                            jt = tch * 4 + ts4
                            psk = psum.tile([128, 256], F32, tag="s_ps", bufs=2)
                            nc.tensor.matmul(
                                psk, ksq[:, ts4 * 128:(ts4 + 1) * 128], ones_m,
                                start=True, stop=True,
                            )
                            col = rk_all[:, ft * 16 + jt:ft * 16 + jt + 1]
                            nc.scalar.activation(
                                col, psk[:, 0:1], AF.Sqrt,
                                scale=1.0 / 128.0, bias=eps_c,
                            )
                            nc.vector.reciprocal(col, col)
                        khat = stg.tile([128, 512], F32R, tag="khat_a", bufs=3)
                        nc.vector.tensor_scalar_mul(
                            khat, st.bitcast(F32), scalar1=wqk_t
                        )
                        nc.sync.dma_start(
                            out=kt_s[ft * 128:(ft + 1) * 128,
                                     tch * 512:(tch + 1) * 512],
                            in_=khat,
                        )

                    # ---- v: natural layout, store
                    for ts4 in range(4):
                        ps = psum.tile([128, 512], F32, tag="big_ps", bufs=2)
                        for ct in range(16):
                            nc.tensor.matmul(
                                ps,
                                x_t[:, ct, ts4 * 128:(ts4 + 1) * 128],
                                wv_t[:, ct, :],
                                start=(ct == 0),
                                stop=(ct == 15),
                            )
                        st = stg.tile([128, 512], F32R, tag="st", bufs=3)
                        nc.scalar.copy(st, ps)
                        nc.sync.dma_start(
                            out=v_s[(tch * 4 + ts4) * 128:(tch * 4 + ts4 + 1) * 128, :],
                            in_=st,
                        )

            # ---------------- Phase B: attention per head -----------------
            with (
                tc.tile_pool(name="head", bufs=1) as head,
                tc.tile_pool(name="ppool", bufs=3) as ppool,
                tc.tile_pool(name="yfin_pool", bufs=1) as yfin_pool,
                tc.tile_pool(name="small", bufs=2) as small,
            ):
                yfin = []
                for h in range(HPG):
                    yf = yfin_pool.tile([128, T], F32R, tag=f"yfin{h}")
                    yfin.append(yf)

                v_sr = v_s.rearrange("(jt p) f -> p jt f", p=128)
                ysum_sb = []  # [1,512] per (h, ic) for deferred subln rstd
                for idx in range(HPG * TC):
                    yt = small.tile([1, 512], F32R, tag=f"ysum{idx}", bufs=1)
                    ysum_sb.append(yt)

                def finish_chunk(y_ps, den_ps, h, ic):
                    # yn = y_unnorm / den  (done via bcast + 128-lane recip);
                    # rstd is deferred to the batched pass after attention.
                    dsb = small.tile([1, 512], F32R, tag="dsb")
                    nc.scalar.copy(dsb, den_ps)
                    db_ps = psum.tile([128, 512], F32, tag="rowb_ps", bufs=2)
                    nc.tensor.matmul(db_ps, onesr_t, dsb, start=True, stop=True)
                    recip_b = small.tile([128, 512], F32, tag="recip_b")
                    nc.vector.reciprocal_approx_fast(recip_b, db_ps)
                    yslice = yfin[h][:, ic * 512:(ic + 1) * 512]
                    nc.vector.tensor_mul(yslice, y_ps, recip_b)
                    ysq = small.tile([128, 512], F32R, tag="ysq")
                    nc.vector.tensor_mul(
                        ysq, yslice.bitcast(F32), yslice.bitcast(F32)
                    )
                    ysum_ps = psum.tile([1, 512], F32, tag="rowb_ps", bufs=2)
                    nc.tensor.matmul(ysum_ps, onesc_t, ysq, start=True, stop=True)
                    nc.scalar.copy(ysum_sb[h * TC + ic], ysum_ps)

                def emit_head_chunk_prep(h, ic, khat_by_h, v_by_h):
                    qhat = small.tile([128, 512], F32R, tag="qhat", bufs=4)
                    nc.sync.dma_start(
                        out=qhat,
                        in_=qt_s[h * 128:(h + 1) * 128,
                                 ic * 512:(ic + 1) * 512],
                    )
                    njt = 4 * ic + 4
                    single_exp = h >= 2
                    vhat = None
                    if single_exp:
                        vhat = head.tile([128, 16, 128], F32R, tag="vhat",
                                         bufs=2)
                        for jt in range(njt):
                            kk = njt - 1 - jt
                            nc.vector.tensor_scalar_mul(
                                vhat[:, jt, :],
                                v_by_h[h][:, jt, :].bitcast(F32),
                                scalar1=dcol_t[:, h * 16 + kk:
                                               h * 16 + kk + 1].bitcast(F32),
                            )
                    y_ps = psum.tile([128, 512], F32, tag="big_ps", bufs=2)
                    den_ps = psum.tile([1, 512], F32, tag="den_ps", bufs=2)
                    return dict(h=h, ic=ic, qhat=qhat, vhat=vhat, y_ps=y_ps,
                                den_ps=den_ps, njt=njt, single_exp=single_exp,
                                khat=khat_by_h[h], v_h=v_by_h[h])

                def emit_jt_iter(st, jt):
                    h, ic = st["h"], st["ic"]
                    rk = rk_all[:, h * 16:(h + 1) * 16]
                    njt = st["njt"]
                    s_ps = psum.tile([128, 512], F32, tag="s_ps", bufs=2)
                    nc.tensor.matmul(
                        s_ps, st["khat"][:, jt * 128:(jt + 1) * 128],
                        st["qhat"], start=True, stop=True,
                    )
                    pt = ppool.tile([128, 512], F32R, tag="pt")
                    i_lo = max(0, jt - 4 * ic)
                    if st["single_exp"]:
                        if i_lo > 0:
                            nc.vector.tensor_scalar_add(
                                s_ps[:, 0:i_lo * 128],
                                s_ps[:, 0:i_lo * 128], scalar1=NEG,
                            )
                        if jt >= 4 * ic:
                            isub = jt - 4 * ic
                            src_b = s_ps[:, isub * 128:(isub + 1) * 128]
                            nc.vector.tensor_add(src_b, src_b, mask_t)
                        nc.scalar.activation(
                            pt, s_ps, AF.Exp,
                            scale=rk[:, jt:jt + 1],
                            bias=bias_t[:, h * 16:h * 16 + 1],
                        )
                        kk = njt - 1 - jt
                        av_lhs = st["vhat"][:, jt, :]
                        den_lhs = dcol_t[:, h * 16 + kk:h * 16 + kk + 1]
                    else:
                        if i_lo > 0:
                            nc.vector.tensor_copy(
                                pt[:, 0:i_lo * 128], zeros_r[:, 0:i_lo * 128]
                            )
                        for isub in range(i_lo, 4):
                            diff = 4 * ic + isub - jt
                            src_b = s_ps[:, isub * 128:(isub + 1) * 128]
                            if diff == 0:
                                nc.vector.tensor_add(src_b, src_b, mask_t)
                            nc.scalar.activation(
                                pt[:, isub * 128:(isub + 1) * 128], src_b,
                                AF.Exp,
                                scale=rk[:, jt:jt + 1],
                                bias=bias_t[:, h * 16 + diff:
                                            h * 16 + diff + 1],
                            )
                        av_lhs = st["v_h"][:, jt, :]
                        den_lhs = onesc_t
                    nc.tensor.matmul(
                        st["y_ps"], av_lhs, pt,
                        start=(jt == 0), stop=(jt == njt - 1),
                        skip_group_check=True,
                    )
                    nc.tensor.matmul(
                        st["den_ps"], den_lhs, pt,
                        start=(jt == 0), stop=(jt == njt - 1),
                        skip_group_check=True,
                    )

                khat_by_h = {}
                v_by_h = {}
                pending = None
                for h in range(HPG):
                    kh = head.tile([128, T], F32R, tag="khat", bufs=2)
                    nc.sync.dma_start(
                        out=kh, in_=kt_s[h * 128:(h + 1) * 128, :]
                    )
                    khat_by_h[h] = kh
                    vh = head.tile([128, 16, 128], F32R, tag="v_h", bufs=2)
                    nc.sync.dma_start(
                        out=vh, in_=v_sr[:, :, h * 128:(h + 1) * 128]
                    )
                    v_by_h[h] = vh
                    for ic in range(TC):
                        st = emit_head_chunk_prep(h, ic, khat_by_h, v_by_h)
                        for jt in range(4 * ic + 4):
                            emit_jt_iter(st, jt)
                        if pending is not None:
                            finish_chunk(pending["y_ps"], pending["den_ps"],
                                         pending["h"], pending["ic"])
                        pending = st
                finish_chunk(pending["y_ps"], pending["den_ps"],
                             pending["h"], pending["ic"])

                # batched subln rstd: one table-switch total, 128-lane recips
                for h in range(HPG):
                    for ic in range(TC):
                        ysb_ps = psum.tile([128, 512], F32, tag="rowb_ps", bufs=2)
                        nc.tensor.matmul(
                            ysb_ps, onesr_t, ysum_sb[h * TC + ic],
                            start=True, stop=True,
                        )
                        srt = small.tile([128, 512], F32, tag="srt")
                        nc.scalar.activation(
                            srt, ysb_ps, AF.Sqrt, scale=1.0 / 128.0, bias=eps_c
                        )
                        srt2 = small.tile([128, 512], F32, tag="srt2")
                        nc.vector.reciprocal_approx_fast(srt2, srt)
                        yslice = yfin[h][:, ic * 512:(ic + 1) * 512]
                        nc.vector.tensor_mul(yslice, yslice.bitcast(F32), srt2)

                # ------------- Phase C: output projection -----------------
                with (
                    tc.tile_pool(name="wppool", bufs=1) as wppool,
                    tc.tile_pool(name="opool", bufs=3) as opool,
                ):
                    wp_t = wppool.tile([128, HPG, C], F32R, tag="wp_t")
                    nc.sync.dma_start(
                        out=wp_t, in_=wp.rearrange("(ht p) c -> p ht c", p=128)
                    )
                    for tt in range(16):
                        for cc in range(4):
                            ps = psum.tile([128, 512], F32, tag="big_ps", bufs=2)
                            for h in range(HPG):
                                nc.tensor.matmul(
                                    ps,
                                    yfin[h][:, tt * 128:(tt + 1) * 128],
                                    wp_t[:, h, cc * 512:(cc + 1) * 512],
                                    start=(h == 0),
                                    stop=(h == HPG - 1),
                                )
                            ot = opool.tile([128, 512], F32, tag="ot")
                            nc.vector.tensor_copy(ot, ps)
                            nc.sync.dma_start(
                                out=out[tt * 128:(tt + 1) * 128,
                                        cc * 512:(cc + 1) * 512],
                                in_=ot,
                            )

    nc.compile()
    return nc


_NC_CACHE = None


def _get_nc():
    global _NC_CACHE
    if _NC_CACHE is None:
        _NC_CACHE = _build()
    return _NC_CACHE


def kernel_in_maps(x, Wq, Wk, Wv, Wproj, q_rms_w, k_rms_w, subln_w):
    slopes = _alibi_slopes(H)

    x = np.asarray(x, dtype=np.float32)
    Wq = np.asarray(Wq, dtype=np.float32)
    Wk = np.asarray(Wk, dtype=np.float32)
    Wv = np.asarray(Wv, dtype=np.float32)
    Wproj = np.asarray(Wproj, dtype=np.float32)
    q_rms_w = np.asarray(q_rms_w, dtype=np.float32)
    k_rms_w = np.asarray(k_rms_w, dtype=np.float32)
    subln_w = np.asarray(subln_w, dtype=np.float32)

    wqk = (q_rms_w * k_rms_w).reshape(128, 1)
    cmask = np.where(
        np.arange(128)[:, None] <= np.arange(128)[None, :], 0.0, NEG
    ).astype(np.float32)
    ones_c = np.ones((128, 1), np.float32)
    ones_r = np.ones((1, 128), np.float32)
    dj = np.arange(128, dtype=np.float32)

    in_maps = []
    for b in range(B):
        xt = np.ascontiguousarray(x[b].T)
        for g in range(HG):
            heads = [g + 4 * j for j in range(HPG)]  # strided: slopes shrink with j
            csel = np.concatenate(
                [np.arange(hh * D, (hh + 1) * D) for hh in heads]
            )
            wproj_s = np.ascontiguousarray(
                Wproj[csel, :] * np.tile(subln_w, HPG)[:, None]
            )
            bias_tab = np.empty((128, HPG * 16), np.float32)
            dcol_a = np.empty((128, HPG * 16), np.float32)
            for j, hh in enumerate(heads):
                slope = slopes[hh]
                for diff in range(16):
                    bias_tab[:, j * 16 + diff] = slope * (dj - PIN - 128.0 * diff)
                    dcol_a[:, j * 16 + diff] = np.float32(
                        math.exp(-slope * 128.0 * diff) if slope * 128.0 * diff < 700
                        else 0.0
                    )
            in_maps.append({
                "xt": xt,
                "wq": np.ascontiguousarray(Wq[:, csel]),
                "wk": np.ascontiguousarray(Wk[:, csel]),
                "wv": np.ascontiguousarray(Wv[:, csel]),
                "wp": wproj_s,
                "bias_tab": bias_tab,
                "cmask": cmask,
                "wqk": wqk,
                "ones_c": ones_c,
                "ones_r": ones_r,
                "dcol": dcol_a,
            })

    return in_maps


def gather(results):
    outs = [r["out"] for r in results]
    final = np.stack(
        [sum(outs[b * HG + 1:(b + 1) * HG], outs[b * HG]) for b in range(B)]
    )
    return final.astype(np.float32)


def kernel(x, Wq, Wk, Wv, Wproj, q_rms_w, k_rms_w, subln_w):
    from concourse.bass_utils import run_bass_kernel_spmd

    in_maps = kernel_in_maps(x, Wq, Wk, Wv, Wproj, q_rms_w, k_rms_w, subln_w)
    res = run_bass_kernel_spmd(_get_nc(), in_maps, core_ids=list(range(8)))
    return gather(res.results)


if __name__ == "__main__":
    rng = np.random.default_rng(0)
    ins = {
        "x": rng.standard_normal((B, T, C), dtype=np.float32),
        "Wq": rng.standard_normal((C, H * D), dtype=np.float32) / math.sqrt(C),
        "Wk": rng.standard_normal((C, H * D), dtype=np.float32) / math.sqrt(C),
        "Wv": rng.standard_normal((C, H * D), dtype=np.float32) / math.sqrt(C),
        "Wproj": rng.standard_normal((H * D, C), dtype=np.float32) * 0.001,
        "q_rms_w": np.ones(D, np.float32),
        "k_rms_w": np.ones(D, np.float32),
        "subln_w": np.ones(D, np.float32),
    }
    y = kernel(**ins)
    print("kernel output", y.shape, y.dtype, float(np.abs(y).mean()))
